# revision 6
# baseline (speedup 1.0000x reference)
"""MoE experts kernel for Trainium2 (Bass/Tile), expert-parallel across 8 NeuronCores.

Problem: nn_CompressedMoeExperts — T=2048 tokens, D=1024, FF=1536, E=8 experts,
top-k=2.  out[t] = sum_e combine[e,t] * (silu(h[t] @ Wg[e].T) * (h[t] @ Wu[e].T)) @ Wd[e].T

Sharding: expert-parallel — core e holds expert e's three weight matrices.
Dispatch (gather of routed tokens) and combine (weighted scatter-add) happen on
the host as part of sharding/unsharding; each core runs a dense 3-matmul MLP on
its routed tokens (padded to a common capacity C) with the combine weight
applied on-device before writeback.

Matmuls run as float32r (full fp32 data, 1 cycle/row on the PE when the moving
dim is >=256), accumulating in fp32 PSUM.  All DMA feeds are pre-laid-out on
the host into the exact SBUF tile layouts so every DMA is contiguous.
"""

import sys

sys.path.insert(0, "/opt/trn_rl_repo")

import numpy as np

import concourse.bass as bass
import concourse.mybir as mybir
import concourse.tile as tile
from concourse import bacc
from concourse.bass_utils import run_bass_kernel_spmd

# Fixed problem shape
T, D, FF, E, TOPK = 2048, 1024, 1536, 8, 2
P = 128
DSUB = D // P     # 8   k-subtiles over the D contraction
FBLK = FF // P    # 12  blocks over the FF dimension
NDN = 512         # free-dim tile for the down projection
NDT = D // NDN    # 2

F32 = mybir.dt.float32
F32R = mybir.dt.float32r

_program_cache: dict[int, "bass.Bass"] = {}
last_results = None  # BassKernelResults of the most recent run (for profiling)


def _chunks(C: int) -> list[int]:
    """Split C (multiple of 128) into matmul moving-dim chunks <=512, all >=256
    when possible (float32r runs 4x slower below 256 moving elements)."""
    nb = C // P
    n = -(-nb * P // 512)  # ceil(C/512)
    base, rem = divmod(nb, n)
    return [(base + (1 if i < rem else 0)) * P for i in range(n)]


def _build_program(C: int) -> "bass.Bass":
    nc = bacc.Bacc(None, target_bir_lowering=False)

    xt_d = nc.dram_tensor("xt", [P, DSUB, C], F32R, kind="ExternalInput")
    wg_d = nc.dram_tensor("wg", [FBLK, P, DSUB, P], F32R, kind="ExternalInput")
    wu_d = nc.dram_tensor("wu", [FBLK, P, DSUB, P], F32R, kind="ExternalInput")
    wd_d = nc.dram_tensor("wd", [P, FBLK, NDT, NDN], F32R, kind="ExternalInput")
    wt_d = nc.dram_tensor("wt", [P, C // P], F32, kind="ExternalInput")
    y_d = nc.dram_tensor("y", [C // P, P, D], F32, kind="ExternalOutput")

    csizes = _chunks(C)

    with tile.TileContext(nc) as tc:
        with (
            tc.tile_pool(name="const", bufs=1) as const_pool,
            tc.tile_pool(name="wpool", bufs=3) as wpool,
            tc.tile_pool(name="actp", bufs=1) as act_pool,
            tc.tile_pool(name="sgp", bufs=3) as sg_pool,
            tc.tile_pool(name="yp", bufs=3) as y_pool,
            tc.tile_pool(name="psum", bufs=2, space="PSUM") as psum_pool,
        ):
            xt = const_pool.tile([P, DSUB, C], F32R)
            nc.sync.dma_start(xt[:], xt_d[:])
            wt_sb = const_pool.tile([P, C // P], F32)
            nc.sync.dma_start(wt_sb[:], wt_d[:])
            wd_sb = const_pool.tile([P, FBLK, NDT, NDN], F32R)
            nc.sync.dma_start(wd_sb[:], wd_d[:])

            # actT[f, fb, t] = silu(gateT) * upT, layout [128, FBLK, C]
            act = act_pool.tile([P, FBLK, C], F32R)

            # Phase 1: gateT/upT = W @ hT per FF-block, fused silu*up
            for fb in range(FBLK):
                wg_t = wpool.tile([P, DSUB, P], F32R, tag="wg")
                nc.sync.dma_start(wg_t[:], wg_d[fb])
                wu_t = wpool.tile([P, DSUB, P], F32R, tag="wu")
                nc.sync.dma_start(wu_t[:], wu_d[fb])
                col = 0
                for cs in csizes:
                    pg = psum_pool.tile([P, NDN], F32, tag="pg", name="pg")[:, :cs]
                    pu = psum_pool.tile([P, NDN], F32, tag="pu", name="pu")[:, :cs]
                    for k in range(DSUB):
                        nc.tensor.matmul(
                            pg,
                            wg_t[:, k, :],
                            xt[:, k, col : col + cs],
                            start=(k == 0),
                            stop=(k == DSUB - 1),
                        )
                    for k in range(DSUB):
                        nc.tensor.matmul(
                            pu,
                            wu_t[:, k, :],
                            xt[:, k, col : col + cs],
                            start=(k == 0),
                            stop=(k == DSUB - 1),
                        )
                    sg = sg_pool.tile([P, NDN], F32, tag="sg", name="sg")[:, :cs]
                    nc.scalar.activation(sg, pg, mybir.ActivationFunctionType.Silu)
                    nc.vector.tensor_mul(act[:, fb, col : col + cs], sg, pu)
                    col += cs

            # Phase 2: y[t, d] = (actT.T @ WdT) * combine_weight[t]
            for tb in range(C // P):
                for dti in range(NDT):
                    py = psum_pool.tile([P, NDN], F32, tag="py")
                    for fs in range(FBLK):
                        nc.tensor.matmul(
                            py,
                            act[:, fs, tb * P : (tb + 1) * P],
                            wd_sb[:, fs, dti, :],
                            start=(fs == 0),
                            stop=(fs == FBLK - 1),
                        )
                    y_sb = y_pool.tile([P, NDN], F32, tag="ysb")
                    nc.vector.tensor_scalar_mul(y_sb, py, wt_sb[:, tb : tb + 1])
                    nc.sync.dma_start(y_d[tb, :, dti * NDN : (dti + 1) * NDN], y_sb)

    nc.compile()
    return nc


def kernel(hidden_states, top_k_index, top_k_weights, gate_proj, up_proj, down_proj):
    global last_results

    h = np.ascontiguousarray(np.asarray(hidden_states, dtype=np.float32))
    idx = np.asarray(top_k_index)
    wts = np.asarray(top_k_weights, dtype=np.float32)
    gp = np.asarray(gate_proj, dtype=np.float32)
    up = np.asarray(up_proj, dtype=np.float32)
    dp = np.asarray(down_proj, dtype=np.float32)
    assert h.shape == (T, D) and idx.shape == (T, TOPK)
    assert gp.shape == (E, FF, D) and dp.shape == (E, D, FF)

    # combine[e, t] = sum_k wts[t, k] * (idx[t, k] == e)
    combine = np.zeros((E, T), np.float32)
    for k in range(TOPK):
        np.add.at(combine, (idx[:, k], np.arange(T)), wts[:, k])

    routed = [np.nonzero(combine[e] > 0)[0] for e in range(E)]
    max_cnt = max(len(r) for r in routed)
    C = max(P, -(-max_cnt // P) * P)

    if C not in _program_cache:
        _program_cache[C] = _build_program(C)
    nc = _program_cache[C]

    in_maps = []
    for e in range(E):
        r = routed[e]
        n_e = len(r)
        idx_pad = np.zeros(C, np.int64)
        idx_pad[:n_e] = r
        wt_pad = np.zeros(C, np.float32)
        wt_pad[:n_e] = combine[e, r]

        xg = h[idx_pad]  # [C, D]
        xt_feed = np.ascontiguousarray(xg.reshape(C, DSUB, P).transpose(2, 1, 0))
        wg_feed = np.ascontiguousarray(
            gp[e].reshape(FBLK, P, DSUB, P).transpose(0, 3, 2, 1)
        )
        wu_feed = np.ascontiguousarray(
            up[e].reshape(FBLK, P, DSUB, P).transpose(0, 3, 2, 1)
        )
        wd_feed = np.ascontiguousarray(
            dp[e].reshape(NDT, NDN, FBLK, P).transpose(3, 2, 0, 1)
        )
        wt_feed = np.ascontiguousarray(wt_pad.reshape(C // P, P).T)
        in_maps.append(
            {"xt": xt_feed, "wg": wg_feed, "wu": wu_feed, "wd": wd_feed, "wt": wt_feed}
        )

    last_results = run_bass_kernel_spmd(nc, in_maps, core_ids=list(range(E)))

    out = np.zeros((T, D), np.float32)
    for e in range(E):
        r = routed[e]
        ye = last_results.results[e]["y"].reshape(C, D)
        out[r] += ye[: len(r)]
    return out


# revision 7
# speedup vs baseline: 1.2369x; 1.2369x over previous
"""MoE experts kernel for Trainium2 (Bass/Tile), expert-parallel across 8 NeuronCores.

Problem: nn_CompressedMoeExperts — T=2048 tokens, D=1024, FF=1536, E=8 experts,
top-k=2.  out[t] = sum_e combine[e,t] * (silu(h[t] @ Wg[e].T) * (h[t] @ Wu[e].T)) @ Wd[e].T

Sharding: expert-parallel — core e holds expert e's three weight matrices.
Dispatch (gather of routed tokens) and combine (weighted scatter-add) happen on
the host as part of sharding/unsharding; each core runs a dense 3-matmul MLP on
its routed tokens (padded to a common capacity C) with the combine weight
applied on-device before writeback.

Matmul operands are fp16 (halves HBM traffic vs fp32, 1 cycle/row on the PE,
fast weight loads — unlike float32r which forces a ~190ns LDWEIGHTS per
matmul), accumulating in fp32 PSUM.  Values here are far inside fp16 range, and
fp16's 10-bit mantissa keeps the L2 relative error at ~5e-4.  All DMA feeds are
pre-laid-out on the host into the exact SBUF tile layouts so every DMA is
contiguous, and split into <=512KB pieces so they spread across DMA queues.
"""

import sys

sys.path.insert(0, "/opt/trn_rl_repo")

import numpy as np

import concourse.bass as bass
import concourse.mybir as mybir
import concourse.tile as tile
from concourse import bacc
from concourse.bass_utils import run_bass_kernel_spmd

# Fixed problem shape
T, D, FF, E, TOPK = 2048, 1024, 1536, 8, 2
P = 128
DSUB = D // P     # 8   k-subtiles over the D contraction
FBLK = FF // P    # 12  blocks over the FF dimension
NDN = 512         # free-dim tile for the down projection
NDT = D // NDN    # 2

F32 = mybir.dt.float32
F16 = mybir.dt.float16

_program_cache: dict[int, "bass.Bass"] = {}
last_results = None  # BassKernelResults of the most recent run (for profiling)


def _chunks(C: int) -> list[int]:
    """Split C (multiple of 128) into matmul moving-dim chunks of <=512
    (PSUM bank limit for fp32 accumulation)."""
    nb = C // P
    n = -(-nb * P // 512)  # ceil(C/512)
    base, rem = divmod(nb, n)
    return [(base + (1 if i < rem else 0)) * P for i in range(n)]


def _build_program(C: int) -> "bass.Bass":
    nc = bacc.Bacc(None, target_bir_lowering=False)

    xt_d = nc.dram_tensor("xt", [P, DSUB, C], F16, kind="ExternalInput")
    wg_d = nc.dram_tensor("wg", [FBLK, P, DSUB, P], F16, kind="ExternalInput")
    wu_d = nc.dram_tensor("wu", [FBLK, P, DSUB, P], F16, kind="ExternalInput")
    wd_d = nc.dram_tensor("wd", [FBLK, P, NDT, NDN], F16, kind="ExternalInput")
    wt_d = nc.dram_tensor("wt", [P, C // P], F32, kind="ExternalInput")
    y_d = nc.dram_tensor("y", [C // P, P, D], F32, kind="ExternalOutput")

    csizes = _chunks(C)

    with tile.TileContext(nc) as tc:
        with (
            tc.tile_pool(name="const", bufs=1) as const_pool,
            tc.tile_pool(name="wpool", bufs=3) as wpool,
            tc.tile_pool(name="actp", bufs=1) as act_pool,
            tc.tile_pool(name="sgp", bufs=3) as sg_pool,
            tc.tile_pool(name="yp", bufs=3) as y_pool,
            tc.tile_pool(name="psum", bufs=2, space="PSUM") as psum_pool,
            tc.tile_pool(name="psum_y", bufs=4, space="PSUM") as psum_y_pool,
        ):
            # First weight block, then xt (split per k-subtile so the pieces
            # spread over DMA queues) — the first matmul group needs all of xt
            # and wg[0], so get those in flight before anything else.
            wg_tiles = []
            wu_tiles = []
            wg_t = wpool.tile([P, DSUB, P], F16, tag="wg", name="wg0")
            nc.sync.dma_start(wg_t[:], wg_d[0])
            wu_t = wpool.tile([P, DSUB, P], F16, tag="wu", name="wu0")
            nc.sync.dma_start(wu_t[:], wu_d[0])
            wg_tiles.append(wg_t)
            wu_tiles.append(wu_t)

            xt = const_pool.tile([P, DSUB, C], F16)
            for k in range(DSUB):
                nc.sync.dma_start(xt[:, k], xt_d[:, k])
            wt_sb = const_pool.tile([P, C // P], F32)
            nc.sync.dma_start(wt_sb[:], wt_d[:])

            # wd lives in SBUF in full; its per-block loads are issued inside
            # the phase-1 loop so they fill otherwise-idle DMA time.
            wd_sb = const_pool.tile([P, FBLK, NDT, NDN], F16)

            # actT[f, fb, t] = silu(gateT) * upT, layout [128, FBLK, C]
            act = act_pool.tile([P, FBLK, C], F16)

            # Phase 1: gateT/upT = W @ hT per FF-block, fused silu*up
            for fb in range(FBLK):
                wg_t = wg_tiles[fb]
                wu_t = wu_tiles[fb]
                if fb + 1 < FBLK:
                    nwg = wpool.tile([P, DSUB, P], F16, tag="wg", name="wg")
                    nc.sync.dma_start(nwg[:], wg_d[fb + 1])
                    nwu = wpool.tile([P, DSUB, P], F16, tag="wu", name="wu")
                    nc.sync.dma_start(nwu[:], wu_d[fb + 1])
                    wg_tiles.append(nwg)
                    wu_tiles.append(nwu)
                # stream one wd block per fb iteration
                nc.sync.dma_start(wd_sb[:, fb], wd_d[fb])

                col = 0
                for cs in csizes:
                    pg = psum_pool.tile([P, NDN], F32, tag="pg", name="pg")[:, :cs]
                    pu = psum_pool.tile([P, NDN], F32, tag="pu", name="pu")[:, :cs]
                    for k in range(DSUB):
                        nc.tensor.matmul(
                            pg,
                            wg_t[:, k, :],
                            xt[:, k, col : col + cs],
                            start=(k == 0),
                            stop=(k == DSUB - 1),
                        )
                    for k in range(DSUB):
                        nc.tensor.matmul(
                            pu,
                            wu_t[:, k, :],
                            xt[:, k, col : col + cs],
                            start=(k == 0),
                            stop=(k == DSUB - 1),
                        )
                    sg = sg_pool.tile([P, NDN], F32, tag="sg", name="sg")[:, :cs]
                    nc.scalar.activation(sg, pg, mybir.ActivationFunctionType.Silu)
                    nc.vector.tensor_mul(act[:, fb, col : col + cs], sg, pu)
                    col += cs

            # Phase 2: y[t, d] = (actT.T @ WdT) * combine_weight[t]
            for tb in range(C // P):
                for dti in range(NDT):
                    py = psum_y_pool.tile([P, NDN], F32, tag="py")
                    for fs in range(FBLK):
                        nc.tensor.matmul(
                            py,
                            act[:, fs, tb * P : (tb + 1) * P],
                            wd_sb[:, fs, dti, :],
                            start=(fs == 0),
                            stop=(fs == FBLK - 1),
                        )
                    y_sb = y_pool.tile([P, NDN], F32, tag="ysb")
                    nc.vector.tensor_scalar_mul(y_sb, py, wt_sb[:, tb : tb + 1])
                    nc.sync.dma_start(y_d[tb, :, dti * NDN : (dti + 1) * NDN], y_sb)

    nc.compile()
    return nc


def kernel(hidden_states, top_k_index, top_k_weights, gate_proj, up_proj, down_proj):
    global last_results

    h = np.ascontiguousarray(np.asarray(hidden_states, dtype=np.float32))
    idx = np.asarray(top_k_index)
    wts = np.asarray(top_k_weights, dtype=np.float32)
    gp = np.asarray(gate_proj, dtype=np.float32)
    up = np.asarray(up_proj, dtype=np.float32)
    dp = np.asarray(down_proj, dtype=np.float32)
    assert h.shape == (T, D) and idx.shape == (T, TOPK)
    assert gp.shape == (E, FF, D) and dp.shape == (E, D, FF)

    # combine[e, t] = sum_k wts[t, k] * (idx[t, k] == e)
    combine = np.zeros((E, T), np.float32)
    for k in range(TOPK):
        np.add.at(combine, (idx[:, k], np.arange(T)), wts[:, k])

    routed = [np.nonzero(combine[e] > 0)[0] for e in range(E)]
    max_cnt = max(len(r) for r in routed)
    C = max(P, -(-max_cnt // P) * P)

    if C not in _program_cache:
        _program_cache[C] = _build_program(C)
    nc = _program_cache[C]

    in_maps = []
    for e in range(E):
        r = routed[e]
        n_e = len(r)
        idx_pad = np.zeros(C, np.int64)
        idx_pad[:n_e] = r
        wt_pad = np.zeros(C, np.float32)
        wt_pad[:n_e] = combine[e, r]

        xg = h[idx_pad].astype(np.float16)  # [C, D]
        xt_feed = np.ascontiguousarray(xg.reshape(C, DSUB, P).transpose(2, 1, 0))
        wg_feed = np.ascontiguousarray(
            gp[e].astype(np.float16).reshape(FBLK, P, DSUB, P).transpose(0, 3, 2, 1)
        )
        wu_feed = np.ascontiguousarray(
            up[e].astype(np.float16).reshape(FBLK, P, DSUB, P).transpose(0, 3, 2, 1)
        )
        # wd_feed[fs, p, dt, dn] = down_proj[e][dt*NDN+dn, fs*P+p]
        wd_feed = np.ascontiguousarray(
            dp[e].astype(np.float16).reshape(NDT, NDN, FBLK, P).transpose(2, 3, 0, 1)
        )
        wt_feed = np.ascontiguousarray(wt_pad.reshape(C // P, P).T)
        in_maps.append(
            {"xt": xt_feed, "wg": wg_feed, "wu": wu_feed, "wd": wd_feed, "wt": wt_feed}
        )

    last_results = run_bass_kernel_spmd(nc, in_maps, core_ids=list(range(E)))

    out = np.zeros((T, D), np.float32)
    for e in range(E):
        r = routed[e]
        ye = last_results.results[e]["y"].reshape(C, D)
        out[r] += ye[: len(r)]
    return out
